# revision 1
# baseline (speedup 1.0000x reference)
"""Expert-parallel MoE MLP kernel for Trainium2 (8 NeuronCores).

Problem: out[b,e,n,d] = gelu(x[b,e] @ w1[e] + b1[e]) @ w2[e] + b2[e]
Shapes: x [2,8,1024,1024] f32, w1 [8,1024,4096], b1 [8,4096],
        w2 [8,4096,1024], b2 [8,1024].

Sharding: expert e -> core e. Each core runs a 2048-token MLP:
  [2048,1024] @ [1024,4096] -> gelu -> @ [4096,1024] -> [2048,1024]

Device-side layout: activations live transposed ([feature, token]) so the
contraction dim is always the SBUF partition dim:
  phase 1: psum[h_tile, t] += w1[d_tile, h_tile].T @ xT[d_tile, t]
  phase 2: psum[d_tile, t] += w2[h_tile, d_tile].T @ hT[h_tile, t]
Host transposes x on the way in and out on the way back (part of
shard/unshard), so the device does zero transposes.

All matmul inputs are bf16 (fp32 PSUM accumulation); GELU (tanh approx,
matching jax.nn.gelu default) fused with the b1 add on ScalarE.
"""

import sys

for _p in ("/opt/trn_rl_repo",):
    if _p not in sys.path:
        sys.path.insert(0, _p)

import numpy as np
import ml_dtypes

from contextlib import ExitStack

import concourse.bass as bass
import concourse.tile as tile
from concourse import bacc, mybir
from concourse.bass import _add_dep_helper
from concourse.bass_utils import run_bass_kernel_spmd

BF16 = mybir.dt.bfloat16
F32 = mybir.dt.float32

# Full-problem constants (hardcoded per harness contract).
B, E, N, D, H = 2, 8, 1024, 1024, 4096
T = B * N          # tokens per expert/core
TBLK = 512         # tokens per block (= one PSUM bank of fp32)
P = 128


def build_nc(t=T, d=D, h=H, tblk=TBLK, act=None, repeats=1,
             ps_bufs=2, act_mode="gelu", phases=(1, 2), x_mode="stream",
             chain_pe=False):
    """Build the per-core Bass program. All cores run this same program on
    different data (SPMD). repeats>1 re-runs the token-block loop (weights
    stay resident) — used only for steady-state timing measurements.
    act_mode: "gelu" | "copy_dve" (diagnostic: replace gelu w/ DVE copy)."""
    if act is None:
        act = mybir.ActivationFunctionType.Gelu_apprx_tanh
    kd = d // P        # contraction tiles for phase 1
    nh = h // P        # h tiles (phase-1 outputs / phase-2 contraction)
    nd = d // P        # d tiles (phase-2 outputs)
    nblk = t // tblk

    nc = bacc.Bacc("TRN2", target_bir_lowering=False)

    xt_hbm = nc.dram_tensor("xt", [d, t], BF16, kind="ExternalInput").ap()
    w1_hbm = nc.dram_tensor("w1", [d, h], BF16, kind="ExternalInput").ap()
    w2_hbm = nc.dram_tensor("w2", [h, d], BF16, kind="ExternalInput").ap()
    b1_hbm = nc.dram_tensor("b1", [nh, P], F32, kind="ExternalInput").ap()
    b2_hbm = nc.dram_tensor("b2", [nd, P], F32, kind="ExternalInput").ap()
    out_hbm = nc.dram_tensor("outT", [d, t], F32, kind="ExternalOutput").ap()

    # [feature, x] views with the 128-partition dim innermost in features.
    xt_v = xt_hbm.rearrange("(kd p) t -> p kd t", p=P)
    w1_v = w1_hbm.rearrange("(kd p) h -> p kd h", p=P)
    w2_v = w2_hbm.rearrange("(kh p) d -> p kh d", p=P)

    with tile.TileContext(nc) as tc, ExitStack() as ctx:
        w1_pool = ctx.enter_context(tc.tile_pool(name="w1", bufs=nh))
        w2_pool = ctx.enter_context(tc.tile_pool(name="w2", bufs=nh))
        x_pool = ctx.enter_context(tc.tile_pool(name="x", bufs=2))
        h_pool = ctx.enter_context(tc.tile_pool(name="h", bufs=nh + 2))
        o_pool = ctx.enter_context(tc.tile_pool(name="o", bufs=4))
        c_pool = ctx.enter_context(tc.tile_pool(name="c", bufs=1))
        ps1 = ctx.enter_context(tc.tile_pool(name="ps1", bufs=ps_bufs, space="PSUM"))
        ps2 = ctx.enter_context(tc.tile_pool(name="ps2", bufs=ps_bufs, space="PSUM"))

        # Biases, resident.
        b1_sb = c_pool.tile([P, nh], F32)
        nc.sync.dma_start(out=b1_sb, in_=b1_hbm.rearrange("t p -> p t"))
        b2_sb = c_pool.tile([P, nd], F32)
        nc.sync.dma_start(out=b2_sb, in_=b2_hbm.rearrange("t p -> p t"))

        # Weights, resident in SBUF for the whole kernel. Chunked DMAs so
        # compute can start as soon as the first chunks land.
        w1_t = []
        for ih in range(nh):
            wt = w1_pool.tile([P, kd, P], BF16)
            nc.sync.dma_start(out=wt, in_=w1_v[:, :, ih * P:(ih + 1) * P])
            w1_t.append(wt)
        w2_t = []
        for ikh in range(nh):
            wt = w2_pool.tile([P, d], BF16)
            nc.sync.dma_start(out=wt, in_=w2_v[:, ikh, :])
            w2_t.append(wt)

        prev_mm = [None]

        def MM(*args, **kwargs):
            bi = nc.tensor.matmul(*args, **kwargs)
            if chain_pe and prev_mm[0] is not None:
                _add_dep_helper(bi.ins, prev_mm[0].ins, sync=False,
                                reason="pe emission order")
            prev_mm[0] = bi
            return bi

        gelu = act
        xt_pre = {}
        if x_mode == "preload":
            for ib in range(nblk):
                xt_pre[ib] = c_pool.tile([P, kd, tblk], BF16,
                                         name=f"xp{ib}", tag=f"xp{ib}")
                nc.sync.dma_start(
                    out=xt_pre[ib],
                    in_=xt_v[:, :, ib * tblk:(ib + 1) * tblk])
        for ib in [i % nblk for i in range(nblk * repeats)]:
            tsl = slice(ib * tblk, (ib + 1) * tblk)
            if x_mode == "preload":
                xt = xt_pre[ib]
            else:
                xt = x_pool.tile([P, kd, tblk], BF16)
                if x_mode == "hwdge":
                    nc.sync.dma_start(out=xt, in_=xt_v[:, :, tsl])
                else:
                    nc.gpsimd.dma_start(out=xt, in_=xt_v[:, :, tsl])

            # phase 1: hT[h_tile] = gelu(w1.T @ xT + b1)
            ht = []
            if 1 in phases:
                for ih in range(nh):
                    ps = ps1.tile([P, tblk], F32)
                    for ik in range(kd):
                        MM(
                            ps, w1_t[ih][:, ik, :], xt[:, ik, :],
                            start=(ik == 0), stop=(ik == kd - 1),
                        )
                    hs = h_pool.tile([P, tblk], BF16)
                    if act_mode == "gelu":
                        nc.scalar.activation(hs, ps, gelu, bias=b1_sb[:, ih:ih + 1])
                    else:
                        nc.vector.tensor_copy(hs, ps)
                    ht.append(hs)
            else:
                # diagnostic: fake hT from xt slices (kd divides nh usage)
                for ih in range(nh):
                    hs = h_pool.tile([P, tblk], BF16)
                    nc.vector.tensor_copy(hs, xt[:, ih % kd, :])
                    ht.append(hs)

            # phase 2: outT[d_tile] = w2.T @ hT + b2
            if 2 in phases:
                for idt in range(nd):
                    ps = ps2.tile([P, tblk], F32)
                    for ikh in range(nh):
                        MM(
                            ps, w2_t[ikh][:, idt * P:(idt + 1) * P], ht[ikh],
                            start=(ikh == 0), stop=(ikh == nh - 1),
                        )
                    ob = o_pool.tile([P, tblk], F32)
                    nc.vector.tensor_scalar_add(ob, ps, b2_sb[:, idt:idt + 1])
                    nc.scalar.dma_start(
                        out=out_hbm[idt * P:(idt + 1) * P, tsl], in_=ob
                    )
            elif 1 in phases:
                # keep outputs observable so phase-1 work isn't dead
                idt = 0
                ob = o_pool.tile([P, tblk], F32)
                nc.vector.tensor_copy(ob, ht[ib % nh])
                nc.scalar.dma_start(
                    out=out_hbm[idt * P:(idt + 1) * P, tsl], in_=ob
                )

    nc.compile()
    return nc


_NC_CACHE = {}


def _get_nc():
    if "nc" not in _NC_CACHE:
        _NC_CACHE["nc"] = build_nc()
    return _NC_CACHE["nc"]


def kernel(x, w1, b1, w2, b2):
    nc = _get_nc()
    bf16 = ml_dtypes.bfloat16
    in_maps = []
    for e in range(E):
        xe = np.asarray(x[:, e], dtype=np.float32).reshape(T, D)
        in_maps.append({
            "xt": np.ascontiguousarray(xe.T).astype(bf16),
            "w1": np.asarray(w1[e], dtype=np.float32).astype(bf16),
            "w2": np.asarray(w2[e], dtype=np.float32).astype(bf16),
            "b1": np.ascontiguousarray(
                np.asarray(b1[e], np.float32).reshape(H // P, P)),
            "b2": np.ascontiguousarray(
                np.asarray(b2[e], np.float32).reshape(D // P, P)),
        })

    res = run_bass_kernel_spmd(nc, in_maps, core_ids=list(range(E)))

    out = np.empty((B, E, N, D), dtype=np.float32)
    for e in range(E):
        ot = np.asarray(res.results[e]["outT"])            # [D, T]
        out[:, e] = ot.T.reshape(B, N, D)
    return out



# revision 7
# speedup vs baseline: 1.0038x; 1.0038x over previous
"""Expert-parallel MoE MLP kernel for Trainium2 (8 NeuronCores).

Problem: out[b,e,n,d] = gelu(x[b,e] @ w1[e] + b1[e]) @ w2[e] + b2[e]
Shapes: x [2,8,1024,1024] f32, w1 [8,1024,4096], b1 [8,4096],
        w2 [8,4096,1024], b2 [8,1024].

Sharding: expert e -> core e. Each core runs a 2048-token MLP:
  [2048,1024] @ [1024,4096] -> gelu -> @ [4096,1024] -> [2048,1024]

Device-side layout: activations live transposed ([feature, token]) so the
contraction dim is always the SBUF partition dim:
  phase 1: psum[h_tile, t] += w1[d_tile, h_tile].T @ xT[d_tile, t]
  phase 2: psum[d_tile, t] += w2[h_tile, d_tile].T @ hT[h_tile, t]
Host transposes x on the way in and out on the way back (part of
shard/unshard), so the device does zero transposes.

All matmul inputs are bf16 (fp32 PSUM accumulation); GELU (tanh approx,
matching jax.nn.gelu default) fused with the b1 add on ScalarE.
"""

import sys

for _p in ("/opt/trn_rl_repo",):
    if _p not in sys.path:
        sys.path.insert(0, _p)

import numpy as np
import ml_dtypes

from contextlib import ExitStack

import concourse.bass as bass
import concourse.tile as tile
from concourse import bacc, mybir
from concourse.bass import _add_dep_helper
from concourse.bass_utils import run_bass_kernel_spmd

BF16 = mybir.dt.bfloat16
F32 = mybir.dt.float32

# Full-problem constants (hardcoded per harness contract).
B, E, N, D, H = 2, 8, 1024, 1024, 4096
T = B * N          # tokens per expert/core
TBLK = 512         # tokens per block (= one PSUM bank of fp32)
P = 128


def build_nc(t=T, d=D, h=H, tblk=TBLK, act=None, repeats=1,
             ps_bufs=2, act_mode="gelu", phases=(1, 2), x_mode="stream",
             chain_pe=False):
    """Build the per-core Bass program. All cores run this same program on
    different data (SPMD). repeats>1 re-runs the token-block loop (weights
    stay resident) — used only for steady-state timing measurements.
    act_mode: "gelu" | "copy_dve" (diagnostic: replace gelu w/ DVE copy)."""
    if act is None:
        act = mybir.ActivationFunctionType.Gelu_apprx_tanh
    kd = d // P        # contraction tiles for phase 1
    nh = h // P        # h tiles (phase-1 outputs / phase-2 contraction)
    nd = d // P        # d tiles (phase-2 outputs)
    nblk = t // tblk

    nc = bacc.Bacc("TRN2", target_bir_lowering=False)

    xt_hbm = nc.dram_tensor("xt", [d, t], BF16, kind="ExternalInput").ap()
    w1_hbm = nc.dram_tensor("w1", [d, h], BF16, kind="ExternalInput").ap()
    w2_hbm = nc.dram_tensor("w2", [h, d], BF16, kind="ExternalInput").ap()
    b1_hbm = nc.dram_tensor("b1", [P, nh], F32, kind="ExternalInput").ap()
    b2_hbm = nc.dram_tensor("b2", [P, nd], F32, kind="ExternalInput").ap()
    out_hbm = nc.dram_tensor("outT", [d, t], F32, kind="ExternalOutput").ap()

    # [feature, x] views with the 128-partition dim innermost in features.
    xt_v = xt_hbm.rearrange("(kd p) t -> p kd t", p=P)
    w1_v = w1_hbm.rearrange("(kd p) h -> p kd h", p=P)
    w2_v = w2_hbm.rearrange("(kh p) d -> p kh d", p=P)

    with tile.TileContext(nc) as tc, ExitStack() as ctx:
        w1_pool = ctx.enter_context(tc.tile_pool(name="w1", bufs=nh))
        w2_pool = ctx.enter_context(tc.tile_pool(name="w2", bufs=nh))
        x_pool = ctx.enter_context(tc.tile_pool(name="x", bufs=2))
        h_pool = ctx.enter_context(tc.tile_pool(name="h", bufs=nh + 2))
        o_pool = ctx.enter_context(tc.tile_pool(name="o", bufs=4))
        c_pool = ctx.enter_context(tc.tile_pool(name="c", bufs=1))
        ps1 = ctx.enter_context(tc.tile_pool(name="ps1", bufs=ps_bufs, space="PSUM"))
        ps2 = ctx.enter_context(tc.tile_pool(name="ps2", bufs=ps_bufs, space="PSUM"))

        # Biases, resident. Host passes them pre-transposed ([P, n] layout)
        # so the DMA is contiguous, and they ride the otherwise-idle vector
        # queue — a transposing 4B-element DMA here used to block the sync
        # queue (and thus all w1 loads) for ~10us at kernel start.
        b1_sb = c_pool.tile([P, nh], F32)
        nc.scalar.dma_start(out=b1_sb, in_=b1_hbm)
        b2_sb = c_pool.tile([P, nd], F32)
        nc.scalar.dma_start(out=b2_sb, in_=b2_hbm)

        # Weights, resident in SBUF for the whole kernel. Chunked DMAs so
        # compute can start as soon as the first chunks land; the first tile
        # is split in two so the very first matmul waits on a half-size DMA.
        w1_t = []
        for ih in range(nh):
            wt = w1_pool.tile([P, kd, P], BF16)
            if ih == 0:
                h2 = kd // 2
                nc.sync.dma_start(out=wt[:, :h2, :], in_=w1_v[:, :h2, :P])
                nc.sync.dma_start(out=wt[:, h2:, :], in_=w1_v[:, h2:, :P])
            else:
                nc.sync.dma_start(out=wt, in_=w1_v[:, :, ih * P:(ih + 1) * P])
            w1_t.append(wt)
        w2_t = []
        for ikh in range(nh):
            wt = w2_pool.tile([P, d], BF16)
            nc.sync.dma_start(out=wt, in_=w2_v[:, ikh, :])
            w2_t.append(wt)

        prev_mm = [None]

        def MM(*args, **kwargs):
            bi = nc.tensor.matmul(*args, **kwargs)
            if chain_pe and prev_mm[0] is not None:
                _add_dep_helper(bi.ins, prev_mm[0].ins, sync=False,
                                reason="pe emission order")
            prev_mm[0] = bi
            return bi

        gelu = act
        xt_pre = {}
        if x_mode == "preload":
            for ib in range(nblk):
                xt_pre[ib] = c_pool.tile([P, kd, tblk], BF16,
                                         name=f"xp{ib}", tag=f"xp{ib}")
                nc.sync.dma_start(
                    out=xt_pre[ib],
                    in_=xt_v[:, :, ib * tblk:(ib + 1) * tblk])
        for ib in [i % nblk for i in range(nblk * repeats)]:
            tsl = slice(ib * tblk, (ib + 1) * tblk)
            if x_mode == "preload":
                xt = xt_pre[ib]
            else:
                xt = x_pool.tile([P, kd, tblk], BF16)
                if x_mode == "hwdge":
                    nc.sync.dma_start(out=xt, in_=xt_v[:, :, tsl])
                elif ib == 0:
                    # Split the first block so the first matmul only waits
                    # on half the transfer.
                    h2 = kd // 2
                    nc.gpsimd.dma_start(out=xt[:, :h2, :], in_=xt_v[:, :h2, tsl])
                    nc.gpsimd.dma_start(out=xt[:, h2:, :], in_=xt_v[:, h2:, tsl])
                else:
                    nc.gpsimd.dma_start(out=xt, in_=xt_v[:, :, tsl])

            # phase 1: hT[h_tile] = gelu(w1.T @ xT + b1)
            ht = []
            if 1 in phases:
                for ih in range(nh):
                    ps = ps1.tile([P, tblk], F32)
                    for ik in range(kd):
                        MM(
                            ps, w1_t[ih][:, ik, :], xt[:, ik, :],
                            start=(ik == 0), stop=(ik == kd - 1),
                        )
                    hs = h_pool.tile([P, tblk], BF16)
                    if act_mode == "gelu":
                        nc.scalar.activation(hs, ps, gelu, bias=b1_sb[:, ih:ih + 1])
                    else:
                        nc.vector.tensor_copy(hs, ps)
                    ht.append(hs)
            else:
                # diagnostic: fake hT from xt slices (kd divides nh usage)
                for ih in range(nh):
                    hs = h_pool.tile([P, tblk], BF16)
                    nc.vector.tensor_copy(hs, xt[:, ih % kd, :])
                    ht.append(hs)

            # phase 2: outT[d_tile] = w2.T @ hT + b2
            if 2 in phases:
                for idt in range(nd):
                    # The very last d-tile of the last block is computed in
                    # two column halves so the kernel-final output DMA is
                    # half-size and overlaps the second half's matmuls.
                    split = 2 if (ib == nblk - 1 and idt == nd - 1) else 1
                    cw = tblk // split
                    for ic in range(split):
                        csl = slice(ic * cw, (ic + 1) * cw)
                        ps = ps2.tile([P, cw], F32)
                        for ikh in range(nh):
                            MM(
                                ps, w2_t[ikh][:, idt * P:(idt + 1) * P],
                                ht[ikh][:, csl],
                                start=(ikh == 0), stop=(ikh == nh - 1),
                            )
                        ob = o_pool.tile([P, cw], F32)
                        nc.vector.tensor_scalar_add(ob, ps, b2_sb[:, idt:idt + 1])
                        nc.scalar.dma_start(
                            out=out_hbm[idt * P:(idt + 1) * P,
                                        ib * tblk + ic * cw:
                                        ib * tblk + (ic + 1) * cw],
                            in_=ob,
                        )
            elif 1 in phases:
                # keep outputs observable so phase-1 work isn't dead
                idt = 0
                ob = o_pool.tile([P, tblk], F32)
                nc.vector.tensor_copy(ob, ht[ib % nh])
                nc.scalar.dma_start(
                    out=out_hbm[idt * P:(idt + 1) * P, tsl], in_=ob
                )

    nc.compile()
    return nc


_NC_CACHE = {}


def _get_nc():
    if "nc" not in _NC_CACHE:
        _NC_CACHE["nc"] = build_nc()
    return _NC_CACHE["nc"]


def make_in_maps(x, w1, b1, w2, b2):
    bf16 = ml_dtypes.bfloat16
    in_maps = []
    for e in range(E):
        xe = np.asarray(x[:, e], dtype=np.float32).reshape(T, D)
        in_maps.append({
            "xt": np.ascontiguousarray(xe.T).astype(bf16),
            "w1": np.asarray(w1[e], dtype=np.float32).astype(bf16),
            "w2": np.asarray(w2[e], dtype=np.float32).astype(bf16),
            # biases pre-transposed to [P, n] so the device DMA is contiguous
            "b1": np.ascontiguousarray(
                np.asarray(b1[e], np.float32).reshape(H // P, P).T),
            "b2": np.ascontiguousarray(
                np.asarray(b2[e], np.float32).reshape(D // P, P).T),
        })
    return in_maps


def kernel(x, w1, b1, w2, b2):
    nc = _get_nc()
    in_maps = make_in_maps(x, w1, b1, w2, b2)

    res = run_bass_kernel_spmd(nc, in_maps, core_ids=list(range(E)))

    out = np.empty((B, E, N, D), dtype=np.float32)
    for e in range(E):
        ot = np.asarray(res.results[e]["outT"])            # [D, T]
        out[:, e] = ot.T.reshape(B, N, D)
    return out



# revision 11
# speedup vs baseline: 1.0081x; 1.0043x over previous
"""Expert-parallel MoE MLP kernel for Trainium2 (8 NeuronCores).

Problem: out[b,e,n,d] = gelu(x[b,e] @ w1[e] + b1[e]) @ w2[e] + b2[e]
Shapes: x [2,8,1024,1024] f32, w1 [8,1024,4096], b1 [8,4096],
        w2 [8,4096,1024], b2 [8,1024].

Sharding: expert e -> core e. Each core runs a 2048-token MLP:
  [2048,1024] @ [1024,4096] -> gelu -> @ [4096,1024] -> [2048,1024]

Device-side layout: activations live transposed ([feature, token]) so the
contraction dim is always the SBUF partition dim:
  phase 1: psum[h_tile, t] += w1[d_tile, h_tile].T @ xT[d_tile, t]
  phase 2: psum[d_tile, t] += w2[h_tile, d_tile].T @ hT[h_tile, t]
Host transposes x on the way in and out on the way back (part of
shard/unshard), so the device does zero transposes.

All matmul inputs are bf16 (fp32 PSUM accumulation); GELU (tanh approx,
matching jax.nn.gelu default) fused with the b1 add on ScalarE.
"""

import sys

for _p in ("/opt/trn_rl_repo",):
    if _p not in sys.path:
        sys.path.insert(0, _p)

import numpy as np
import ml_dtypes

from contextlib import ExitStack

import concourse.bass as bass
import concourse.tile as tile
from concourse import bacc, mybir
from concourse.bass import _add_dep_helper
from concourse.bass_utils import run_bass_kernel_spmd

BF16 = mybir.dt.bfloat16
F32 = mybir.dt.float32

# Full-problem constants (hardcoded per harness contract).
B, E, N, D, H = 2, 8, 1024, 1024, 4096
T = B * N          # tokens per expert/core
TBLK = 512         # tokens per block (= one PSUM bank of fp32)
P = 128


def build_nc(t=T, d=D, h=H, tblk=TBLK, act=None, repeats=1,
             ps_bufs=2, act_mode="gelu", phases=(1, 2), x_mode="stream",
             chain_pe=False, warm=7):
    """Build the per-core Bass program. All cores run this same program on
    different data (SPMD). repeats>1 re-runs the token-block loop (weights
    stay resident) — used only for steady-state timing measurements.
    act_mode: "gelu" | "copy_dve" (diagnostic: replace gelu w/ DVE copy)."""
    if act is None:
        act = mybir.ActivationFunctionType.Gelu_apprx_tanh
    kd = d // P        # contraction tiles for phase 1
    nh = h // P        # h tiles (phase-1 outputs / phase-2 contraction)
    nd = d // P        # d tiles (phase-2 outputs)
    nblk = t // tblk

    nc = bacc.Bacc("TRN2", target_bir_lowering=False)

    xt_hbm = nc.dram_tensor("xt", [d, t], BF16, kind="ExternalInput").ap()
    w1_hbm = nc.dram_tensor("w1", [d, h], BF16, kind="ExternalInput").ap()
    w2_hbm = nc.dram_tensor("w2", [h, d], BF16, kind="ExternalInput").ap()
    b1_hbm = nc.dram_tensor("b1", [P, nh], F32, kind="ExternalInput").ap()
    b2_hbm = nc.dram_tensor("b2", [P, nd], F32, kind="ExternalInput").ap()
    out_hbm = nc.dram_tensor("outT", [d, t], F32, kind="ExternalOutput").ap()

    # [feature, x] views with the 128-partition dim innermost in features.
    xt_v = xt_hbm.rearrange("(kd p) t -> p kd t", p=P)
    w1_v = w1_hbm.rearrange("(kd p) h -> p kd h", p=P)
    w2_v = w2_hbm.rearrange("(kh p) d -> p kh d", p=P)

    with tile.TileContext(nc) as tc, ExitStack() as ctx:
        w1_pool = ctx.enter_context(tc.tile_pool(name="w1", bufs=nh))
        w2_pool = ctx.enter_context(tc.tile_pool(name="w2", bufs=nh))
        # bufs=1 on x: block ib+1's DMA then waits until block ib's tile is
        # fully consumed by phase 1, keeping the 1MB transfer out of the
        # bandwidth-critical kernel head (it has a ~50us idle window).
        x_pool = ctx.enter_context(tc.tile_pool(name="x", bufs=1))
        h_pool = ctx.enter_context(tc.tile_pool(name="h", bufs=nh + 2))
        o_pool = ctx.enter_context(tc.tile_pool(name="o", bufs=4))
        c_pool = ctx.enter_context(tc.tile_pool(name="c", bufs=1))
        ps1 = ctx.enter_context(
            tc.tile_pool(name="ps1", bufs=ps_bufs + 1, space="PSUM"))
        ps2 = ctx.enter_context(tc.tile_pool(name="ps2", bufs=ps_bufs, space="PSUM"))

        # PE clock warm-up: the PE p-state needs ~3us of continuous busy to
        # reach full clock. Run a few dummy matmuls on scratch data while the
        # head DMAs are in flight so the real matmuls start at full speed.
        if warm:
            wm_pool = ctx.enter_context(tc.tile_pool(name="wm", bufs=1))
            ps_w = ctx.enter_context(tc.tile_pool(name="psw", bufs=1, space="PSUM"))
            wm_w = wm_pool.tile([P, P], BF16)
            wm_x = wm_pool.tile([P, tblk], BF16)
            nc.vector.memset(wm_w, 0)
            nc.vector.memset(wm_x, 0)
            wm_ps = ps_w.tile([P, tblk], F32)
            for _ in range(warm):
                nc.tensor.matmul(wm_ps, wm_w, wm_x, start=True, stop=True)

        # Block-0 input, first: the kernel head is HBM-bandwidth-bound, so
        # block 0's 1MB is split into 4 chunks spread over the gpsimd AND
        # scalar DMA queues (w1 owns the sync queue) to maximize parallel
        # delivery before the first matmuls consume it.
        xt0 = None
        if x_mode == "stream" and 1 in phases:
            xt0 = x_pool.tile([P, kd, tblk], BF16)
            nq = 4 if kd % 4 == 0 else (2 if kd % 2 == 0 else 1)
            q = kd // nq
            for ic in range(nq):
                eng = nc.gpsimd if ic < (nq + 1) // 2 else nc.scalar
                eng.dma_start(out=xt0[:, ic * q:(ic + 1) * q, :],
                              in_=xt_v[:, ic * q:(ic + 1) * q, 0:tblk])

        # Biases, resident. Host passes them pre-transposed ([P, n] layout)
        # so the DMA is contiguous — a transposing 4B-element DMA here used
        # to block the sync queue (and thus all w1 loads) for ~10us at
        # kernel start. They ride the scalar queue behind block-0's x.
        b1_sb = c_pool.tile([P, nh], F32)
        nc.scalar.dma_start(out=b1_sb, in_=b1_hbm)
        b2_sb = c_pool.tile([P, nd], F32)
        nc.scalar.dma_start(out=b2_sb, in_=b2_hbm)

        # Weights, resident in SBUF for the whole kernel. Chunked DMAs so
        # compute can start as soon as the first chunks land; the first tile
        # is split in two so the very first matmul waits on a half-size DMA.
        w1_t = []
        for ih in range(nh):
            wt = w1_pool.tile([P, kd, P], BF16)
            if ih == 0:
                h2 = kd // 2
                nc.sync.dma_start(out=wt[:, :h2, :], in_=w1_v[:, :h2, :P])
                nc.sync.dma_start(out=wt[:, h2:, :], in_=w1_v[:, h2:, :P])
            else:
                nc.sync.dma_start(out=wt, in_=w1_v[:, :, ih * P:(ih + 1) * P])
            w1_t.append(wt)
        w2_t = []
        for ikh in range(nh):
            wt = w2_pool.tile([P, d], BF16)
            nc.sync.dma_start(out=wt, in_=w2_v[:, ikh, :])
            w2_t.append(wt)

        prev_mm = [None]

        def MM(*args, **kwargs):
            bi = nc.tensor.matmul(*args, **kwargs)
            if chain_pe and prev_mm[0] is not None:
                _add_dep_helper(bi.ins, prev_mm[0].ins, sync=False,
                                reason="pe emission order")
            prev_mm[0] = bi
            return bi

        gelu = act
        xt_pre = {}
        if x_mode == "preload":
            for ib in range(nblk):
                xt_pre[ib] = c_pool.tile([P, kd, tblk], BF16,
                                         name=f"xp{ib}", tag=f"xp{ib}")
                nc.sync.dma_start(
                    out=xt_pre[ib],
                    in_=xt_v[:, :, ib * tblk:(ib + 1) * tblk])
        for ib in [i % nblk for i in range(nblk * repeats)]:
            tsl = slice(ib * tblk, (ib + 1) * tblk)
            if x_mode == "preload":
                xt = xt_pre[ib]
            elif x_mode == "stream" and ib == 0 and xt0 is not None:
                xt = xt0
            else:
                xt = x_pool.tile([P, kd, tblk], BF16)
                if x_mode == "hwdge":
                    nc.sync.dma_start(out=xt, in_=xt_v[:, :, tsl])
                else:
                    nc.gpsimd.dma_start(out=xt, in_=xt_v[:, :, tsl])

            # phase 1: hT[h_tile] = gelu(w1.T @ xT + b1)
            ht = []
            if 1 in phases:
                for ih in range(nh):
                    ps = ps1.tile([P, tblk], F32)
                    for ik in range(kd):
                        MM(
                            ps, w1_t[ih][:, ik, :], xt[:, ik, :],
                            start=(ik == 0), stop=(ik == kd - 1),
                        )
                    hs = h_pool.tile([P, tblk], BF16)
                    if act_mode == "gelu":
                        nc.scalar.activation(hs, ps, gelu, bias=b1_sb[:, ih:ih + 1])
                    else:
                        nc.vector.tensor_copy(hs, ps)
                    ht.append(hs)
            else:
                # diagnostic: fake hT from xt slices (kd divides nh usage)
                for ih in range(nh):
                    hs = h_pool.tile([P, tblk], BF16)
                    nc.vector.tensor_copy(hs, xt[:, ih % kd, :])
                    ht.append(hs)

            # phase 2: outT[d_tile] = w2.T @ hT + b2
            if 2 in phases:
                for idt in range(nd):
                    # The very last d-tile of the last block is computed in
                    # two column halves so the kernel-final output DMA is
                    # half-size and overlaps the second half's matmuls.
                    split = 2 if (ib == nblk - 1 and idt == nd - 1) else 1
                    cw = tblk // split
                    for ic in range(split):
                        csl = slice(ic * cw, (ic + 1) * cw)
                        ps = ps2.tile([P, cw], F32)
                        for ikh in range(nh):
                            MM(
                                ps, w2_t[ikh][:, idt * P:(idt + 1) * P],
                                ht[ikh][:, csl],
                                start=(ikh == 0), stop=(ikh == nh - 1),
                            )
                        ob = o_pool.tile([P, cw], F32)
                        nc.vector.tensor_scalar_add(ob, ps, b2_sb[:, idt:idt + 1])
                        nc.scalar.dma_start(
                            out=out_hbm[idt * P:(idt + 1) * P,
                                        ib * tblk + ic * cw:
                                        ib * tblk + (ic + 1) * cw],
                            in_=ob,
                        )
            elif 1 in phases:
                # keep outputs observable so phase-1 work isn't dead
                idt = 0
                ob = o_pool.tile([P, tblk], F32)
                nc.vector.tensor_copy(ob, ht[ib % nh])
                nc.scalar.dma_start(
                    out=out_hbm[idt * P:(idt + 1) * P, tsl], in_=ob
                )

    nc.compile()
    return nc


_NC_CACHE = {}


def _get_nc():
    if "nc" not in _NC_CACHE:
        _NC_CACHE["nc"] = build_nc()
    return _NC_CACHE["nc"]


def make_in_maps(x, w1, b1, w2, b2):
    bf16 = ml_dtypes.bfloat16
    in_maps = []
    for e in range(E):
        xe = np.asarray(x[:, e], dtype=np.float32).reshape(T, D)
        in_maps.append({
            "xt": np.ascontiguousarray(xe.T).astype(bf16),
            "w1": np.asarray(w1[e], dtype=np.float32).astype(bf16),
            "w2": np.asarray(w2[e], dtype=np.float32).astype(bf16),
            # biases pre-transposed to [P, n] so the device DMA is contiguous
            "b1": np.ascontiguousarray(
                np.asarray(b1[e], np.float32).reshape(H // P, P).T),
            "b2": np.ascontiguousarray(
                np.asarray(b2[e], np.float32).reshape(D // P, P).T),
        })
    return in_maps


def kernel(x, w1, b1, w2, b2):
    nc = _get_nc()
    in_maps = make_in_maps(x, w1, b1, w2, b2)

    res = run_bass_kernel_spmd(nc, in_maps, core_ids=list(range(E)))

    out = np.empty((B, E, N, D), dtype=np.float32)
    for e in range(E):
        ot = np.asarray(res.results[e]["outT"])            # [D, T]
        out[:, e] = ot.T.reshape(B, N, D)
    return out

